# revision 4
# baseline (speedup 1.0000x reference)
"""Trainium2 Bass kernel for batched dot-product attention (decode-style).

Computation (per batch element b):
    scores[s] = dot(decoder_state[b], encoder_outputs[b, s])   # [S]
    attn      = softmax(scores)                                 # [S]
    context   = sum_s attn[s] * encoder_outputs[b, s]           # [H]

Shapes: decoder_state [64, 1024] f32, encoder_outputs [64, 2048, 1024] f32.
Returns (context [64, 1024], attn_weights [64, 2048]) matching the reference.

Sharding: batch dim across the 8 NeuronCores (8 batches per core), no
cross-core communication. Per core the encoder slice is 64 MiB, read from
HBM exactly once (memory roofline ~186 us/core at ~360 GB/s):

  per b (pipelined across b via Tile pools):
    - 16 DMA loads of [128 s, 1024 h] tiles, kept SBUF-resident
    - scores: one fused DVE tensor_tensor_reduce per tile
        (elementwise mul with partition-broadcast decoder vector + row sum)
    - softmax: gpsimd partition_all_reduce (cross-partition max / sum),
        ACT Exp with fused accumulate, DVE reciprocal + scalar mul
    - context: PE matmuls, weight column [128,1] stationary, resident enc
        tile streaming, accumulated in PSUM as [1, 512] x 2
    - attn out: PE transpose [128,16] -> [16,128] for contiguous DMA
"""

import numpy as np

B, S, H = 64, 2048, 1024
NCORES = 8
BL = B // NCORES          # 8 batches per core
P = 128                   # partitions per tile
T = S // P                # 16 s-tiles per batch
NH = 512                  # fp32 matmul moving-operand max free dim

_CACHE = {}


def _build_program():
    import concourse.bass as bass
    import concourse.bacc as bacc
    import concourse.tile as tile
    import concourse.mybir as mybir

    f32 = mybir.dt.float32

    nc = bacc.Bacc(
        "TRN2",
        target_bir_lowering=False,
        debug=False,
        num_devices=NCORES,
    )

    dec = nc.dram_tensor("decoder_state", [BL, H], f32, kind="ExternalInput").ap()
    enc = nc.dram_tensor("encoder_outputs", [BL, S, H], f32, kind="ExternalInput").ap()
    ident = nc.dram_tensor("identity", [P, P], f32, kind="ExternalInput").ap()
    ctx_out = nc.dram_tensor("context", [BL, H], f32, kind="ExternalOutput").ap()
    attn_out = nc.dram_tensor("attn_weights", [BL, S], f32, kind="ExternalOutput").ap()

    attn_tiled = attn_out.rearrange("b (t p) -> b t p", p=P)  # [BL, T, P]

    with tile.TileContext(nc) as tc:
        with (
            tc.tile_pool(name="singles", bufs=1) as singles,
            tc.tile_pool(name="enc", bufs=2 * T + 2) as enc_pool,
            tc.tile_pool(name="work", bufs=2) as work,
            tc.tile_pool(name="psum", bufs=2, space="PSUM") as psum,
        ):
            ident_sb = singles.tile([P, P], f32, name="ident_sb")
            nc.sync.dma_start(out=ident_sb, in_=ident)

            for b in range(BL):
                # Broadcast this batch's decoder vector to all 128 partitions.
                dec_bc = work.tile([P, H], f32, name="dec_bc", tag="dec_bc")
                nc.gpsimd.dma_start(
                    out=dec_bc, in_=dec[b : b + 1, :].to_broadcast([P, H])
                )

                # Stream the 16 encoder tiles; fused mul+rowsum -> scores col t.
                # scalar_tensor_tensor: out = (in0 * 1.0) * in1, accum = rowsum.
                # The elementwise product is discarded into a stride-0 dummy
                # sink (same trick as kernels/qr.py) -- only accum_out matters.
                scores = work.tile([P, T], f32, name="scores", tag="scores")
                enc_tiles = []
                for t in range(T):
                    enc_t = enc_pool.tile([P, H], f32, name="enc_t")
                    nc.sync.dma_start(out=enc_t, in_=enc[b, t * P : (t + 1) * P, :])
                    dummy = work.tile([P, 1], f32, name="dummy", tag="dummy")
                    nc.vector.scalar_tensor_tensor(
                        out=dummy.broadcast_to([P, H]),
                        in0=enc_t,
                        scalar=1.0,
                        in1=dec_bc,
                        op0=mybir.AluOpType.mult,
                        op1=mybir.AluOpType.mult,
                        accum_out=scores[:, t : t + 1],
                    )
                    enc_tiles.append(enc_t)

                # Softmax over all 2048 scores of batch b.
                # Cross-partition max (replicated), then negated free-axis max.
                pmax = work.tile([P, T], f32, name="pmax", tag="pmax")
                nc.gpsimd.partition_all_reduce(
                    pmax, scores, channels=P, reduce_op=bass.bass_isa.ReduceOp.max
                )
                negmax = work.tile([P, 1], f32, name="negmax", tag="negmax")
                nc.vector.reduce_max(
                    negmax, pmax, axis=mybir.AxisListType.X, negate=True
                )
                # e = exp(scores - max), rowsum fused.
                e_sb = work.tile([P, T], f32, name="e_sb", tag="e_sb")
                rowsum = work.tile([P, 1], f32, name="rowsum", tag="rowsum")
                nc.scalar.activation(
                    e_sb,
                    scores,
                    mybir.ActivationFunctionType.Exp,
                    bias=negmax,
                    accum_out=rowsum,
                )
                dall = work.tile([P, 1], f32, name="dall", tag="dall")
                nc.gpsimd.partition_all_reduce(
                    dall, rowsum, channels=P, reduce_op=bass.bass_isa.ReduceOp.add
                )
                rd = work.tile([P, 1], f32, name="rd", tag="rd")
                nc.vector.reciprocal(rd, dall)
                wnorm = work.tile([P, T], f32, name="wnorm", tag="wnorm")
                nc.vector.tensor_scalar_mul(wnorm, e_sb, rd)

                # attn_weights out: transpose [128, 16] -> [16, 128] on PE,
                # then 16 contiguous 512 B rows to HBM.
                wt_ps = psum.tile([T, P], f32, name="wt_ps", tag="wt_ps")
                nc.tensor.transpose(wt_ps, wnorm, ident_sb)
                wt_sb = work.tile([T, P], f32, name="wt_sb", tag="wt_sb")
                nc.scalar.activation(
                    wt_sb, wt_ps, mybir.ActivationFunctionType.Copy
                )
                nc.sync.dma_start(out=attn_tiled[b], in_=wt_sb)

                # Context: per tile, weight column stationary, enc tile moving.
                ctx_lo = psum.tile([1, NH], f32, name="ctx_lo", tag="ctx_lo")
                ctx_hi = psum.tile([1, NH], f32, name="ctx_hi", tag="ctx_hi")
                for t in range(T):
                    nc.tensor.matmul(
                        ctx_lo,
                        lhsT=wnorm[:, t : t + 1],
                        rhs=enc_tiles[t][:, 0:NH],
                        start=(t == 0),
                        stop=(t == T - 1),
                    )
                    nc.tensor.matmul(
                        ctx_hi,
                        lhsT=wnorm[:, t : t + 1],
                        rhs=enc_tiles[t][:, NH:H],
                        start=(t == 0),
                        stop=(t == T - 1),
                    )
                ctx_sb = work.tile([1, H], f32, name="ctx_sb", tag="ctx_sb")
                nc.scalar.activation(
                    ctx_sb[:, 0:NH], ctx_lo, mybir.ActivationFunctionType.Copy
                )
                nc.scalar.activation(
                    ctx_sb[:, NH:H], ctx_hi, mybir.ActivationFunctionType.Copy
                )
                nc.sync.dma_start(out=ctx_out[b : b + 1, :], in_=ctx_sb)

    nc.compile()
    return nc


def _get_program():
    if "nc" not in _CACHE:
        _CACHE["nc"] = _build_program()
    return _CACHE["nc"]


def kernel(decoder_state: np.ndarray, encoder_outputs: np.ndarray):
    from concourse.bass_utils import run_bass_kernel_spmd

    nc = _get_program()

    decoder_state = np.ascontiguousarray(np.asarray(decoder_state, dtype=np.float32))
    encoder_outputs = np.ascontiguousarray(
        np.asarray(encoder_outputs, dtype=np.float32)
    )
    ident = np.eye(P, dtype=np.float32)

    in_maps = []
    for c in range(NCORES):
        lo, hi = c * BL, (c + 1) * BL
        in_maps.append(
            {
                "decoder_state": decoder_state[lo:hi],
                "encoder_outputs": encoder_outputs[lo:hi],
                "identity": ident,
            }
        )

    res = run_bass_kernel_spmd(nc, in_maps, list(range(NCORES))).results
    context = np.concatenate([res[c]["context"] for c in range(NCORES)], axis=0)
    attn = np.concatenate([res[c]["attn_weights"] for c in range(NCORES)], axis=0)
    return context, attn


# revision 7
# speedup vs baseline: 223.5032x; 223.5032x over previous
"""Trainium2 Bass kernel for batched dot-product attention (decode-style).

Computation (per batch element b):
    scores[s] = dot(decoder_state[b], encoder_outputs[b, s])   # [S]
    attn      = softmax(scores)                                 # [S]
    context   = sum_s attn[s] * encoder_outputs[b, s]           # [H]

Shapes: decoder_state [64, 1024] f32, encoder_outputs [64, 2048, 1024] f32.
Returns (context [64, 1024], attn_weights [64, 2048]) matching the reference.

Sharding: batch dim across the 8 NeuronCores (8 batches per core), no
cross-core communication. Per core the encoder slice is 64 MiB, read from
HBM exactly once (memory roofline ~186 us/core at ~360 GB/s):

  per b (pipelined across b via Tile pools):
    - 16 DMA loads of [128 s, 1024 h] tiles, kept SBUF-resident
    - scores: one fused DVE tensor_tensor_reduce per tile
        (elementwise mul with partition-broadcast decoder vector + row sum)
    - softmax: gpsimd partition_all_reduce (cross-partition max / sum),
        ACT Exp with fused accumulate, DVE reciprocal + scalar mul
    - context: PE matmuls, weight column [128,1] stationary, resident enc
        tile streaming, accumulated in PSUM as [1, 512] x 2
    - attn out: PE transpose [128,16] -> [16,128] for contiguous DMA
"""

import numpy as np

B, S, H = 64, 2048, 1024
NCORES = 8
BL = B // NCORES          # 8 batches per core
P = 128                   # partitions per tile
T = S // P                # 16 s-tiles per batch
NH = 512                  # fp32 matmul moving-operand max free dim

_CACHE = {}


def _build_program(repeats: int = 1):
    import concourse.bass as bass
    import concourse.bacc as bacc
    import concourse.tile as tile
    import concourse.mybir as mybir

    f32 = mybir.dt.float32

    nc = bacc.Bacc(
        "TRN2",
        target_bir_lowering=False,
        debug=False,
        num_devices=NCORES,
    )

    dec = nc.dram_tensor("decoder_state", [BL, H], f32, kind="ExternalInput").ap()
    enc = nc.dram_tensor("encoder_outputs", [BL, S, H], f32, kind="ExternalInput").ap()
    ident = nc.dram_tensor("identity", [P, P], f32, kind="ExternalInput").ap()
    ctx_out = nc.dram_tensor("context", [BL, H], f32, kind="ExternalOutput").ap()
    attn_out = nc.dram_tensor("attn_weights", [BL, S], f32, kind="ExternalOutput").ap()

    attn_tiled = attn_out.rearrange("b (t p) -> b t p", p=P)  # [BL, T, P]

    with tile.TileContext(nc) as tc:
        with (
            tc.tile_pool(name="singles", bufs=1) as singles,
            tc.tile_pool(name="enc", bufs=2 * T + 2) as enc_pool,
            tc.tile_pool(name="work", bufs=2) as work,
            tc.tile_pool(name="psum", bufs=2, space="PSUM") as psum,
        ):
            ident_sb = singles.tile([P, P], f32, name="ident_sb")
            nc.sync.dma_start(out=ident_sb, in_=ident)

            for _rep in range(repeats):
              for b in range(BL):
                # Broadcast this batch's decoder vector to all 128 partitions.
                dec_bc = work.tile([P, H], f32, name="dec_bc", tag="dec_bc")
                nc.gpsimd.dma_start(
                    out=dec_bc, in_=dec[b : b + 1, :].to_broadcast([P, H])
                )

                # Stream the 16 encoder tiles; fused mul+rowsum -> scores col t.
                # scalar_tensor_tensor: out = (in0 * 1.0) * in1, accum = rowsum.
                # The elementwise product is discarded into a stride-0 dummy
                # sink (same trick as kernels/qr.py) -- only accum_out matters.
                scores = work.tile([P, T], f32, name="scores", tag="scores")
                enc_tiles = []
                for t in range(T):
                    enc_t = enc_pool.tile([P, H], f32, name="enc_t")
                    nc.sync.dma_start(out=enc_t, in_=enc[b, t * P : (t + 1) * P, :])
                    dummy = work.tile([P, 1], f32, name="dummy", tag="dummy")
                    nc.vector.scalar_tensor_tensor(
                        out=dummy.broadcast_to([P, H]),
                        in0=enc_t,
                        scalar=1.0,
                        in1=dec_bc,
                        op0=mybir.AluOpType.mult,
                        op1=mybir.AluOpType.mult,
                        accum_out=scores[:, t : t + 1],
                    )
                    enc_tiles.append(enc_t)

                # Softmax over all 2048 scores of batch b.
                # Cross-partition max (replicated), then negated free-axis max.
                pmax = work.tile([P, T], f32, name="pmax", tag="pmax")
                nc.gpsimd.partition_all_reduce(
                    pmax, scores, channels=P, reduce_op=bass.bass_isa.ReduceOp.max
                )
                negmax = work.tile([P, 1], f32, name="negmax", tag="negmax")
                nc.vector.reduce_max(
                    negmax, pmax, axis=mybir.AxisListType.X, negate=True
                )
                # e = exp(scores - max), rowsum fused.
                e_sb = work.tile([P, T], f32, name="e_sb", tag="e_sb")
                rowsum = work.tile([P, 1], f32, name="rowsum", tag="rowsum")
                nc.scalar.activation(
                    e_sb,
                    scores,
                    mybir.ActivationFunctionType.Exp,
                    bias=negmax,
                    accum_out=rowsum,
                )
                dall = work.tile([P, 1], f32, name="dall", tag="dall")
                nc.gpsimd.partition_all_reduce(
                    dall, rowsum, channels=P, reduce_op=bass.bass_isa.ReduceOp.add
                )
                rd = work.tile([P, 1], f32, name="rd", tag="rd")
                nc.vector.reciprocal(rd, dall)
                wnorm = work.tile([P, T], f32, name="wnorm", tag="wnorm")
                nc.vector.tensor_scalar_mul(wnorm, e_sb, rd)

                # attn_weights out: transpose [128, 16] -> [16, 128] on PE,
                # then 16 contiguous 512 B rows to HBM.
                wt_ps = psum.tile([T, P], f32, name="wt_ps", tag="wt_ps")
                nc.tensor.transpose(wt_ps, wnorm, ident_sb)
                wt_sb = work.tile([T, P], f32, name="wt_sb", tag="wt_sb")
                nc.scalar.activation(
                    wt_sb, wt_ps, mybir.ActivationFunctionType.Copy
                )
                nc.sync.dma_start(out=attn_tiled[b], in_=wt_sb)

                # Context: per tile, weight column stationary, enc tile moving.
                ctx_lo = psum.tile([1, NH], f32, name="ctx_lo", tag="ctx_lo")
                ctx_hi = psum.tile([1, NH], f32, name="ctx_hi", tag="ctx_hi")
                for t in range(T):
                    nc.tensor.matmul(
                        ctx_lo,
                        lhsT=wnorm[:, t : t + 1],
                        rhs=enc_tiles[t][:, 0:NH],
                        start=(t == 0),
                        stop=(t == T - 1),
                    )
                    nc.tensor.matmul(
                        ctx_hi,
                        lhsT=wnorm[:, t : t + 1],
                        rhs=enc_tiles[t][:, NH:H],
                        start=(t == 0),
                        stop=(t == T - 1),
                    )
                ctx_sb = work.tile([1, H], f32, name="ctx_sb", tag="ctx_sb")
                nc.scalar.activation(
                    ctx_sb[:, 0:NH], ctx_lo, mybir.ActivationFunctionType.Copy
                )
                nc.scalar.activation(
                    ctx_sb[:, NH:H], ctx_hi, mybir.ActivationFunctionType.Copy
                )
                nc.sync.dma_start(out=ctx_out[b : b + 1, :], in_=ctx_sb)

    nc.compile()
    return nc


def _get_program(repeats: int = 1):
    key = ("nc", repeats)
    if key not in _CACHE:
        _CACHE[key] = _build_program(repeats)
    return _CACHE[key]


def kernel(decoder_state: np.ndarray, encoder_outputs: np.ndarray):
    from concourse.bass_utils import run_bass_kernel_spmd

    nc = _get_program()

    decoder_state = np.ascontiguousarray(np.asarray(decoder_state, dtype=np.float32))
    encoder_outputs = np.ascontiguousarray(
        np.asarray(encoder_outputs, dtype=np.float32)
    )
    ident = np.eye(P, dtype=np.float32)

    in_maps = []
    for c in range(NCORES):
        lo, hi = c * BL, (c + 1) * BL
        in_maps.append(
            {
                "decoder_state": decoder_state[lo:hi],
                "encoder_outputs": encoder_outputs[lo:hi],
                "identity": ident,
            }
        )

    res = run_bass_kernel_spmd(nc, in_maps, list(range(NCORES))).results
    context = np.concatenate([res[c]["context"] for c in range(NCORES)], axis=0)
    attn = np.concatenate([res[c]["attn_weights"] for c in range(NCORES)], axis=0)
    return context, attn
